# revision 20
# baseline (speedup 1.0000x reference)
"""Multi-head attention block (nn_Attention) on 8 Trainium2 NeuronCores.

Reference computation (fp32):
    qkv = x @ w_qkv;  q,k,v = split(qkv);  per-head softmax(q k^T / sqrt(d)) v
    out = concat_heads @ w_out + b_out
Shapes: x [4, 2048, 1024], w_qkv [1024, 3072], w_out [1024, 1024], b_out [1024].

Sharding: DP over batch (4) x TP over head-groups (2) = 8 cores.
Core c handles batch c//2 and heads [8*(c%2), 8*(c%2)+8). Each core computes a
partial output projection over its 8 heads; the host sums the two partials per
batch and adds b_out (the unshard/gather step). No on-device collectives.

Performance design (derived from NTFF trace analysis of the v1 kernel):
  The attention middle is ScalarE-bound: 256 EXP activations x ~1.34us = 342us
  is the hard floor (exp rate is 1 elem/lane/cycle @1.2GHz; PSUM bank geometry
  caps each activation at [128,1024]). Everything else must hide under it:
  - All matmul operands are fp16 (x and the weights are cast host-side; TRN2
    forbids mixing 32/16-bit matmul inputs). Activation/weight magnitudes are
    O(1), so fp16's 0.05% rounding is negligible against the 2e-2 gate.
  - x is shipped pre-transposed (fp16) from the host, so x^T lands in SBUF
    via plain contiguous DMAs: no PE transposes, no staging tiles, no
    evacuation copies. (The X-bar DMA-transpose path silently corrupts
    strided DRAM column-slice sources, so it is not used.)
  - PE warm-up matmuls at t=0 keep the HAM clock gate from running the head
    phase at 1.2 GHz (it otherwise unthrottles only ~25us in).
  - Output projection is woven into pair 3's attention loop per-ib and the
    result DMA'd out immediately, killing the 70us serial tail.
  - Softmax denominators use reciprocal_approx_fast (custom DVE op, ~5x
    faster than nc.vector.reciprocal which cost 127us of DVE time).
  - All PSUM->SBUF staging copies in the attention window run on DVE/GpSimd,
    never ScalarE.

Per-core kernel:
  head:  x^T via contiguous DMA (pre-transposed host-side); V = x w_v
         (+ones col -> V_aug, fp16); Q^T/K^T chunks for head-pair 0
  attn:  per head-pair: S^T = K^T.T Q^T (scores transposed so the softmax
         axis lands on the PSUM free dim; the two heads' matmuls row-tile
         into the upper/lower 64 rows of the PE array and run concurrently);
         P^T = exp(S^T/8) on ScalarE (no max subtraction needed: |S/8| < ~6);
         O_aug^T = V_aug^T P^T accumulated in PSUM, row 64 = softmax
         denominators; normalization on DVE + GpSimd. The QKV projection
         matmuls for the NEXT head pair are interleaved into the attention
         loop; pair 3 instead weaves the output projection.
"""
import sys

sys.path.insert(0, "/opt/trn_rl_repo")

import numpy as np

import concourse.bacc as bacc
import concourse.mybir as mybir
from concourse import masks
from concourse.tile import TileContext
from concourse.bass_utils import run_bass_kernel_spmd

F32 = mybir.dt.float32
F32R = mybir.dt.float32r
F16 = mybir.dt.float16
EXP = mybir.ActivationFunctionType.Exp

T = 2048      # tokens per core (one batch element)
E = 1024      # model dim
HPC = 8       # heads per core
D = 64        # head dim
SCALE = D ** -0.5
NEC = E // 128   # 8 e-chunks
NI = 4           # i blocks of 512 (attention query cols)
NJ = 16          # j blocks of 128 (attention key rows = t blocks)

_CACHED_NC = None


def build_nc():
    nc = bacc.Bacc("TRN2", target_bir_lowering=False, debug=False, num_devices=8)
    x_d = nc.declare_dram_parameter("x", [E, T], F16, isOutput=False)  # x^T (host)
    wqk_d = nc.declare_dram_parameter("wqk", [E, 1024], F16, isOutput=False)
    wv_d = nc.declare_dram_parameter("wv", [E, 512], F16, isOutput=False)
    wo_d = nc.declare_dram_parameter("wo", [512, E], F16, isOutput=False)
    out_d = nc.declare_dram_parameter("out", [T, E], F32, isOutput=True)

    with TileContext(nc) as tc:
        with (
            tc.tile_pool(name="const", bufs=1) as const_pool,
            tc.tile_pool(name="qkt", bufs=2) as qkt_pool,
            tc.tile_pool(name="vaugp", bufs=1) as vaug_pool,
            tc.tile_pool(name="xph", bufs=1) as x_pool,
            tc.tile_pool(name="wstr", bufs=4) as w_pool,
            tc.tile_pool(name="wvp", bufs=1) as wv_pool,
        ):
            identF = const_pool.tile([128, 128], F32, tag="identF")
            masks.make_identity(nc, identF[:])
            ident = const_pool.tile([128, 128], F32R, tag="ident")
            nc.vector.tensor_copy(ident[:], identF[:])
            onesF = const_pool.tile([128, 64], F32, tag="onesF")
            nc.vector.memset(onesF[:], 1.0)

            vaug = [
                vaug_pool.tile([128, HPC * 65], F16, tag=f"va{jb}", name=f"va{jb}")
                for jb in range(NJ)
            ]
            xT = [
                x_pool.tile([128, T], F16, tag=f"xT{ec}", name=f"xT{ec}")
                for ec in range(NEC)
            ]

            def qk_pair_closures(pair, qp, kp, psum_ref):
                """Closures computing Q^T/K^T chunks for head pair `pair` into
                qp/kp (fp16). First closure prefetches the weights. psum_ref is
                a 1-element list so deferred closures allocate from whichever
                PSUM pool is open when they actually run."""
                wq = w_pool.tile([128, E], F16, tag="wcb", name=f"wq{pair}")
                wk = w_pool.tile([128, E], F16, tag="wcb", name=f"wk{pair}")
                cls = []

                def load_w():
                    for ec in range(NEC):
                        nc.gpsimd.dma_start(
                            out=wq[:, ec * 128 : (ec + 1) * 128],
                            in_=wqk_d[
                                ec * 128 : (ec + 1) * 128,
                                pair * 128 : (pair + 1) * 128,
                            ],
                        )
                        nc.gpsimd.dma_start(
                            out=wk[:, ec * 128 : (ec + 1) * 128],
                            in_=wqk_d[
                                ec * 128 : (ec + 1) * 128,
                                512 + pair * 128 : 512 + (pair + 1) * 128,
                            ],
                        )

                cls.append(load_w)
                for wcb, dst in ((wq, qp), (wk, kp)):
                    for ib in range(NI):
                        # each accumulation chain is split into 2-matmul
                        # quanta so a woven chunk never delays the S->exp
                        # pipeline by more than ~0.4us
                        box = [None]
                        for e0 in range(0, NEC, 2):
                            def part(wcb=wcb, dst=dst, ib=ib, box=box, e0=e0):
                                if e0 == 0:
                                    box[0] = psum_ref[0].tile(
                                        [128, 512], F32, tag="qkp",
                                        name="qkchunk",
                                    )
                                ps = box[0]
                                for ec in (e0, e0 + 1):
                                    nc.tensor.matmul(
                                        ps[:],
                                        wcb[:, ec * 128 : (ec + 1) * 128],
                                        xT[ec][:, ib * 512 : (ib + 1) * 512],
                                        start=(ec == 0),
                                        stop=(ec == NEC - 1),
                                    )
                                if e0 == NEC - 2:
                                    nc.vector.tensor_copy(
                                        dst[:, ib * 512 : (ib + 1) * 512], ps[:]
                                    )

                            cls.append(part)
                return cls

            wv_sb = wv_pool.tile([128, NEC * 512], F16, tag="wv")
            vq_psum_ref = [None]

            def v_closures(jb):
                """V projection for j-block jb, split into 2-matmul quanta
                (last part evacuates PSUM into vaug[jb])."""
                box = [None]
                parts = []
                for e0 in range(0, NEC, 2):
                    def part(jb=jb, box=box, e0=e0):
                        if e0 == 0:
                            box[0] = vq_psum_ref[0].tile(
                                [128, 512], F32, tag="qkp", name="vchunk"
                            )
                        ps = box[0]
                        for ec in (e0, e0 + 1):
                            nc.tensor.matmul(
                                ps[:],
                                xT[ec][:, jb * 128 : (jb + 1) * 128],
                                wv_sb[:, ec * 512 : (ec + 1) * 512],
                                start=(ec == 0),
                                stop=(ec == NEC - 1),
                            )
                        if e0 == NEC - 2:
                            vview = vaug[jb][:].rearrange("p (h c) -> p h c", c=65)
                            nc.vector.tensor_copy(
                                vview[:, :, 0:64],
                                ps[:].rearrange("p (h c) -> p h c", c=64),
                            )

                    parts.append(part)
                return parts

            # ---------------- head phase: x^T, V_aug[0], qk pair 0 ---------
            with (
                tc.tile_pool(name="warm_ps", bufs=1, space="PSUM") as warm_psum,
                tc.tile_pool(name="qk_ps", bufs=3, space="PSUM") as qk_psum,
            ):
                # PE warm-up: dummy matmuls spanning t=0 .. ~11us (the x^T
                # DMA window) so the HAM clock gate reaches K=8/8 before the
                # real head-phase matmuls run AND stays there (a single short
                # burst misses the 4096-cycle activity window).
                warm = warm_psum.tile([128, 128], F32, tag="warm")
                for wi in range(110):
                    nc.tensor.matmul(
                        warm[:], ident[:], ident[:],
                        start=(wi == 0), stop=(wi == 109),
                    )
                vq_psum_ref[0] = qk_psum

                # x^T is shipped pre-transposed from the host, so the load is
                # 8 plain contiguous [128, 2048] DMAs at full bandwidth.
                for ec in range(NEC):
                    eng = nc.sync if ec % 2 == 0 else nc.scalar
                    eng.dma_start(
                        out=xT[ec][:],
                        in_=x_d[ec * 128 : (ec + 1) * 128, :],
                    )
                for ec in range(NEC):
                    nc.gpsimd.dma_start(
                        out=wv_sb[:, ec * 512 : (ec + 1) * 512],
                        in_=wv_d[ec * 128 : (ec + 1) * 128, :],
                    )
                for jb in range(NJ):
                    vview = vaug[jb][:].rearrange("p (h c) -> p h c", c=65)
                    nc.vector.tensor_copy(
                        vview[:, :, 64:65],
                        onesF[:, 0:HPC].rearrange("p (h c) -> p h c", c=1),
                    )
                for jb in range(NJ):
                    for fn in v_closures(jb):
                        fn()

                # qk chunks for pair 0: weights, q-ib0 and all k-groups
                # upfront (every k column is swept within the first j-loop);
                # q-ib1..3 deferred into hc0's attention loop.
                qp0 = qkt_pool.tile([128, T], F16, tag="qp", name="qp0")
                kp0 = qkt_pool.tile([128, T], F16, tag="kp", name="kp0")
                qk0_psum_ref = [qk_psum]
                cls0 = qk_pair_closures(0, qp0, kp0, qk0_psum_ref)
                # layout: [load_w, q-ib0..3 (4 parts each), k-ib0..3 (4 each)]
                cls0[0]()          # load_w
                for fn in cls0[1:5]:
                    fn()           # q-ib0
                for fn in cls0[17:33]:
                    fn()           # k-ib0..3
                qk0_deferred = cls0[5:17]

            # ---------------- attention + woven QKV / out-projection -------
            with (
                tc.tile_pool(name="otp", bufs=1) as ot_pool,
                tc.tile_pool(name="wop", bufs=1) as wo_pool,
            ):
                oT = [
                    ot_pool.tile([128, T], F16, tag=f"oT{hc}", name=f"oT{hc}")
                    for hc in range(4)
                ]
                wo_sb = [
                    wo_pool.tile([128, E], F16, tag=f"wo{hc}", name=f"wo{hc}")
                    for hc in range(4)
                ]
                for hc in range(4):
                    nc.gpsimd.dma_start(
                        out=wo_sb[hc][:], in_=wo_d[hc * 128 : (hc + 1) * 128, :]
                    )

                with (
                    tc.tile_pool(name="pt", bufs=3) as pt_pool,
                    tc.tile_pool(name="ocp", bufs=3) as oc_pool,
                    tc.tile_pool(name="rbp", bufs=3) as rb_pool,
                    tc.tile_pool(name="ost", bufs=3) as ost_pool,
                    tc.tile_pool(name="s_ps", bufs=2, space="PSUM") as s_psum,
                    tc.tile_pool(name="oa_ps", bufs=2, space="PSUM") as oa_psum,
                    tc.tile_pool(name="qk3_ps", bufs=2, space="PSUM") as qk3_psum,
                ):
                    def proj_closures(ib):
                        """Output projection + DMA for the 512 tokens of ib.
                        Runs only after all four pairs' oT cover those tokens
                        (i.e. woven into pair 3 after its ib completes)."""
                        cls = []
                        for tch in range(4):
                            for eb in range(2):
                                box = [None]
                                for h0 in (0, 2):
                                    def part(tch=tch, eb=eb, box=box, h0=h0):
                                        trows = slice(
                                            ib * 512 + tch * 128,
                                            ib * 512 + (tch + 1) * 128,
                                        )
                                        ecols = slice(eb * 512, (eb + 1) * 512)
                                        if h0 == 0:
                                            box[0] = qk3_psum.tile(
                                                [128, 512], F32, tag="qkp",
                                                name="prjchunk",
                                            )
                                        ps = box[0]
                                        for hc in (h0, h0 + 1):
                                            nc.tensor.matmul(
                                                ps[:],
                                                oT[hc][:, trows],
                                                wo_sb[hc][:, ecols],
                                                start=(hc == 0),
                                                stop=(hc == 3),
                                            )
                                        if h0 == 2:
                                            ot = ost_pool.tile(
                                                [128, 512], F32, tag="ost"
                                            )
                                            nc.vector.tensor_copy(ot[:], ps[:])
                                            eng = (
                                                nc.sync
                                                if (tch + eb) % 2 == 0
                                                else nc.gpsimd
                                            )
                                            eng.dma_start(
                                                out=out_d[trows, ecols], in_=ot[:]
                                            )

                                    cls.append(part)
                        return cls

                    qp, kp = qp0, kp0
                    pending = []
                    for hc in range(4):
                        hA, hB = 2 * hc, 2 * hc + 1
                        if hc < 3:
                            qn = qkt_pool.tile([128, T], F16, tag="qp", name=f"qp{hc+1}")
                            kn = qkt_pool.tile([128, T], F16, tag="kp", name=f"kp{hc+1}")
                            pending = qk_pair_closures(hc + 1, qn, kn, [qk3_psum])
                            if hc == 0:
                                qk0_psum_ref[0] = qk3_psum
                                vq_psum_ref[0] = qk3_psum
                                pending = qk0_deferred + pending
                        else:
                            qn = kn = None
                        for ib in range(NI):
                            if hc == 3 and ib > 0:
                                # weave the previous ib's output projection
                                pending = pending + proj_closures(ib - 1)
                            icols = slice(ib * 512, (ib + 1) * 512)
                            oaugA = oa_psum.tile([65, 512], F32, tag="oa", name="oaugA")
                            oaugB = oa_psum.tile([65, 512], F32, tag="oa", name="oaugB")
                            prev_pAB = None

                            def emit_pv(pAB, jb):
                                nc.tensor.matmul(
                                    oaugA[:],
                                    vaug[jb][:, hA * 65 : hA * 65 + 65],
                                    pAB[:, 0:512],
                                    start=(jb == 0), stop=(jb == NJ - 1),
                                )
                                nc.tensor.matmul(
                                    oaugB[:],
                                    vaug[jb][:, hB * 65 : hB * 65 + 65],
                                    pAB[:, 512:1024],
                                    start=(jb == 0), stop=(jb == NJ - 1),
                                )

                            for jb in range(NJ):
                                jcols = slice(jb * 128, (jb + 1) * 128)
                                sAB = s_psum.tile([128, 1024], F32, tag="sAB")
                                nc.tensor.matmul(
                                    sAB[:, 0:512], kp[0:64, jcols], qp[0:64, icols],
                                    start=True, stop=True,
                                )
                                nc.tensor.matmul(
                                    sAB[:, 512:1024], kp[64:128, jcols],
                                    qp[64:128, icols],
                                    start=True, stop=True,
                                )
                                pAB = pt_pool.tile([128, 1024], F16, tag="pAB")
                                nc.scalar.activation(pAB[:], sAB[:], EXP, scale=SCALE)
                                if prev_pAB is not None:
                                    emit_pv(prev_pAB, jb - 1)
                                prev_pAB = pAB
                                # weave deferred V / next pair's QKV (or pair
                                # 3's output projection) into spare PE
                                # cycles in small quanta; pair 0's first ib
                                # drains aggressively because V rides just
                                # ahead of its PV consumers
                                if pending:
                                    pending.pop(0)()
                            emit_pv(prev_pAB, NJ - 1)

                            for oaug, rowoff in ((oaugA, 0), (oaugB, 64)):
                                oc = oc_pool.tile([64, 512], F32, tag="oc")
                                nc.vector.tensor_copy(oc[:], oaug[0:64, :])
                                # denominator row staged to partition 0: the
                                # custom-DVE reciprocal misreads inputs whose
                                # AP base partition is nonzero
                                den0 = oc_pool.tile([1, 512], F32, tag="den0")
                                nc.vector.tensor_copy(den0[0:1, :], oaug[64:65, :])
                                rc0 = oc_pool.tile([1, 512], F32, tag="rc0")
                                nc.vector.reciprocal_approx_fast(
                                    out=rc0[0:1, :], in_=den0[0:1, :]
                                )
                                rbs = rb_pool.tile([64, 512], F32, tag="rbs")
                                nc.gpsimd.partition_broadcast(rbs[:], rc0[0:1, :])
                                nc.vector.tensor_mul(
                                    oT[hc][rowoff : rowoff + 64, icols],
                                    oc[:],
                                    rbs[:],
                                )
                        for fn in pending:
                            fn()
                        pending = []
                        qp, kp = qn, kn

                    # last ib's output projection (everything else is done)
                    for fn in proj_closures(NI - 1):
                        fn()

    nc.compile()
    return nc


def get_nc():
    global _CACHED_NC
    if _CACHED_NC is None:
        _CACHED_NC = build_nc()
    return _CACHED_NC


def make_in_maps(x, w_qkv, w_out):
    in_maps = []
    for c in range(8):
        bi, hg = divmod(c, 2)
        wqk_c = np.concatenate(
            [
                w_qkv[:, hg * 512 : hg * 512 + 512],
                w_qkv[:, 1024 + hg * 512 : 1024 + hg * 512 + 512],
            ],
            axis=1,
        )
        in_maps.append(
            {
                "x": np.ascontiguousarray(x[bi].T.astype(np.float16)),
                "wqk": np.ascontiguousarray(wqk_c).astype(np.float16),
                "wv": np.ascontiguousarray(
                    w_qkv[:, 2048 + hg * 512 : 2048 + hg * 512 + 512]
                ).astype(np.float16),
                "wo": np.ascontiguousarray(
                    w_out[hg * 512 : hg * 512 + 512, :]
                ).astype(np.float16),
            }
        )
    return in_maps


def kernel(x, w_qkv, w_out, b_out):
    x = np.asarray(x, dtype=np.float32)
    w_qkv = np.asarray(w_qkv, dtype=np.float32)
    w_out = np.asarray(w_out, dtype=np.float32)
    b_out = np.asarray(b_out, dtype=np.float32)
    nc = get_nc()
    res = run_bass_kernel_spmd(nc, make_in_maps(x, w_qkv, w_out), list(range(8)))
    parts = [res.results[c]["out"] for c in range(8)]
    out = np.stack([parts[2 * bi] + parts[2 * bi + 1] for bi in range(4)])
    out += b_out[None, None, :]
    return out.astype(np.float32)
